# revision 2
# baseline (speedup 1.0000x reference)
"""DilateAttention Trainium2 kernel (nn_DilateAttention).

Full inputs q,k,v: [8, 192, 56, 56] fp32. Output: [8, 56, 56, 192] fp32.
Sharded data-parallel over batch B=8 across 8 NeuronCores.

v4: single uniform pipeline of 54 units (3 slices x 2 pixel-quarters x 9
shifts). The score matmul uses a same-head 0/1 selector [128->128] that
REDUCES over head_dim and BROADCASTS to all channels in one PE pass, so
scores are born channel-broadcast:

  unit (slice, qi, j):
    prod  = q * k_shift            (DVE/GpSimd, bf16, shared by qi pair)
    S_bc  = selG @ prod            (PE -> PSUM [128,784], fp32)
    E_bc  = exp(SCALE * S_bc)      (Act -> SBUF bf16; evacuation fused)
    prod2 = E_bc * v_shift         (DVE/GpSimd, bf16 2x)
    ACC  += I @ prod2              (PE, PSUM accumulate over j)
    D    += selD(1/32) @ E_bc      (PE, PSUM accumulate over j -> [4,784])

  per (slice, qi) tail:
    out_un = copy(ACC)             (Act -> bf16, frees ACC early)
    D_sb   = copy(D)               (DVE -> bf16, frees D)
    D_bc   = selNB @ D_sb          (PE -> PSUM [128,784])
    out    = out_un / D_bc         (DVE divide, 1x)
    DMA out -> o[channel-major]

Softmax normalization thus happens at the very end (numerator and
denominator both accumulate in fp32 PSUM). No compact softmax stage, no
attn re-broadcast pass, no PSUM->SBUF copies of attention weights.

PSUM (8 banks): S_bc [128,784]x2 bufs = 4, ACC x1 = 2, D/D_bc x1 = 2.
Host stages padded/duplicated bf16 inputs; output is channel-major bf16,
host-transposed to [H, W, C] fp32.
"""

import sys

for _p in ("/opt/trn_rl_repo",):
    if _p not in sys.path:
        sys.path.insert(0, _p)

import ml_dtypes
import numpy as np

BF = ml_dtypes.bfloat16

B = 8
C = 192
H = W = 56
HD = 32
KK = 9
SCALE = HD ** -0.5
HWPIX = H * W  # 3136
HALF = HWPIX // 2  # 1568
QTR = HALF // 2  # 784
HROWS = H // 2  # 28
QROWS = HROWS // 2  # 14
SHIFTS = [(di, dj) for di in (-2, 0, 2) for dj in (-2, 0, 2)]

PADH = PADW = 60
ROW0 = COL0 = 2
PADH1 = 32

# ---- engine assignment (tunable) ----
A_POOL = {(0, 8), (1, 8), (2, 8)}
DO_D = True
S_BUFS = 2
TAIL_LAG = 5
N_WARMUP = 0


def c_strategy(i, qi, j):
    """Which engine runs the attn*v multiply: 'dve' or 'pool'."""
    if j in (2, 6):
        return "pool"
    return "dve"


def _sub2(p):
    return ((p % 64) // HD) * 2 + p // 64


def _build_consts():
    consts = {}
    g = np.zeros((128, 128), np.float32)
    for p in range(128):
        for d in range(128):
            if p // HD == d // HD:
                g[p, d] = 1.0
    consts["selG0"] = g
    g = np.zeros((128, 128), np.float32)
    for p in range(128):
        for d in range(128):
            if _sub2(p) == _sub2(d):
                g[p, d] = 1.0
    consts["selG2"] = g
    d0 = np.zeros((128, 4), np.float32)
    for p in range(128):
        d0[p, p // HD] = 1.0 / HD
    consts["selD0"] = d0
    d2 = np.zeros((128, 4), np.float32)
    for p in range(128):
        d2[p, _sub2(p)] = 1.0 / HD
    consts["selD2"] = d2
    nb = np.zeros((4, 128), np.float32)
    for d in range(128):
        nb[d // HD, d] = 1.0
    consts["selNB0"] = nb
    nb = np.zeros((4, 128), np.float32)
    for d in range(128):
        nb[_sub2(d), d] = 1.0
    consts["selNB2"] = nb
    consts["ident"] = np.eye(128, dtype=np.float32)
    return consts


def _bank_chunks(c0, c1):
    out = []
    while c0 < c1:
        nxt = min((c0 // 512 + 1) * 512, c1)
        out.append((c0, nxt))
        c0 = nxt
    return out


def build_module():
    import concourse.bacc as bacc
    import concourse.mybir as mybir
    import concourse.tile as tile

    fp32 = mybir.dt.float32
    bf = mybir.dt.bfloat16
    AL = mybir.AluOpType

    nc = bacc.Bacc("TRN2", target_bir_lowering=False, debug=False, num_devices=B)

    q0_d = nc.dram_tensor("q0", [128, H, W], bf, kind="ExternalInput")
    q1_d = nc.dram_tensor("q1", [128, HROWS, W], bf, kind="ExternalInput")
    k0_d = nc.dram_tensor("k0", [128, PADH, PADW], bf, kind="ExternalInput")
    v0_d = nc.dram_tensor("v0", [128, PADH, PADW], bf, kind="ExternalInput")
    k1_d = nc.dram_tensor("k1", [128, PADH1, PADW], bf, kind="ExternalInput")
    v1_d = nc.dram_tensor("v1", [128, PADH1, PADW], bf, kind="ExternalInput")
    o_d = nc.dram_tensor("o", [C, HWPIX], bf, kind="ExternalOutput")
    consts = _build_consts()
    c_d = {
        name: nc.dram_tensor(name, list(arr.shape), bf, kind="ExternalInput")
        for name, arr in consts.items()
    }

    with tile.TileContext(nc) as tc:
        with (
            tc.tile_pool(name="io", bufs=1) as io_pool,
            tc.tile_pool(name="work", bufs=2) as work_pool,
            tc.tile_pool(name="small", bufs=1) as small_pool,
            tc.tile_pool(name="psS", bufs=S_BUFS, space="PSUM") as psS,
            tc.tile_pool(name="psACC", bufs=1, space="PSUM") as psACC,
            tc.tile_pool(name="psD", bufs=1, space="PSUM") as psD,
        ):
            sel_sb = {}

            def load_const(name):
                arr = consts[name]
                t = small_pool.tile(list(arr.shape), bf, tag=f"c_{name}", name=f"c_{name}")
                nc.sync.dma_start(t[:], c_d[name][:])
                sel_sb[name] = t

            def load_img(name, src_d, shape):
                t = io_pool.tile(list(shape), bf, tag=name, name=name)
                nc.sync.dma_start(t[:], src_d[:])
                return t

            q1 = load_img("q1", q1_d, [128, HROWS, W])
            k1 = load_img("k1", k1_d, [128, PADH1, PADW])
            load_const("selG2")
            v1 = load_img("v1", v1_d, [128, PADH1, PADW])
            load_const("selD2")
            load_const("ident")
            load_const("selNB2")
            q0 = load_img("q0", q0_d, [128, H, W])
            k0 = load_img("k0", k0_d, [128, PADH, PADW])
            load_const("selG0")
            load_const("selD0")
            load_const("selNB0")
            v0 = load_img("v0", v0_d, [128, PADH, PADW])

            slices = []
            for i in range(3):
                if i < 2:
                    slices.append(
                        dict(
                            q=q0, kt=k0, vt=v0, rbase=ROW0 + i * HROWS,
                            qrow=i * HROWS, selG="selG0", selD="selD0",
                            selNB="selNB0", ocn=128, ocol=i * HALF,
                        )
                    )
                else:
                    slices.append(
                        dict(
                            q=q1, kt=k1, vt=v1, rbase=ROW0, qrow=0,
                            selG="selG2", selD="selD2", selNB="selNB2",
                            ocn=64, ocol=0,
                        )
                    )

            prods = [[None] * KK for _ in range(3)]

            if N_WARMUP:
                # PE clock-ramp warmup: back-to-back dummy matmuls during
                # the input-DMA prologue (results never read).
                warm = psS.tile([128, QTR], fp32, tag="S", name="warm")
                for _ in range(N_WARMUP):
                    nc.tensor.matmul(
                        warm[:, 0:128],
                        sel_sb["selG2"][:],
                        sel_sb["selG2"][:],
                        start=True,
                        stop=True,
                    )

            def emit_prod(i, j):
                s = slices[i]
                di, dj = SHIFTS[j]
                p = work_pool.tile(
                    [128, HROWS, W], bf, tag=f"pr{i}", bufs=KK, name=f"pr{i}_{j}"
                )
                kv = s["kt"][
                    :,
                    s["rbase"] + di : s["rbase"] + di + HROWS,
                    COL0 + dj : COL0 + dj + W,
                ]
                qv = s["q"][:, s["qrow"] : s["qrow"] + HROWS, :]
                eng = nc.gpsimd if (i, j) in A_POOL else nc.vector
                eng.tensor_tensor(p[:], qv, kv, AL.mult)
                prods[i][j] = p

            # ---- global unit stream with lagged tail ops ----
            units = []
            for i in [2, 0, 1]:
                for qi in (0, 1):
                    for j in range(KK):
                        units.append((i, qi, j))

            acc_t = {}
            d_t = {}
            pending = []

            def emit_head(i, qi, j):
                s = slices[i]
                di, dj = SHIFTS[j]
                if qi == 0:
                    emit_prod(i, j)
                pf = prods[i][j].rearrange("p a b -> p (a b)")
                S_t = psS.tile([128, QTR], fp32, tag="S", name=f"S{i}{qi}{j}")
                for c0, c1 in _bank_chunks(0, QTR):
                    nc.tensor.matmul(
                        S_t[:, c0:c1],
                        sel_sb[s["selG"]][:],
                        pf[:, qi * QTR + c0 : qi * QTR + c1],
                        start=True,
                        stop=True,
                    )
                E_u = work_pool.tile(
                    [128, QTR], bf, tag="eb", bufs=12, name=f"E{i}{qi}{j}"
                )
                nc.scalar.activation(
                    E_u[:],
                    S_t[:],
                    mybir.ActivationFunctionType.Exp,
                    scale=float(SCALE),
                )
                r0 = s["rbase"] + di + qi * QROWS
                vv = s["vt"][:, r0 : r0 + QROWS, COL0 + dj : COL0 + dj + W]
                prod2 = work_pool.tile(
                    [128, QROWS, W], bf, tag="cp", bufs=12, name=f"cp{i}{qi}{j}"
                )
                m_eng = nc.gpsimd if c_strategy(i, qi, j) == "pool" else nc.vector
                m_eng.tensor_tensor(
                    prod2[:],
                    E_u.rearrange("p (a b) -> p a b", a=QROWS),
                    vv,
                    AL.mult,
                )
                return E_u, prod2

            def emit_tail_ops(i, qi, j, E_u, prod2):
                if j == 0:
                    acc_t[(i, qi)] = psACC.tile(
                        [128, QTR], fp32, tag="ACC", name=f"ACC{i}{qi}"
                    )
                    if DO_D:
                        d_t[(i, qi)] = psD.tile(
                            [128, QTR], fp32, tag="D", name=f"D{i}{qi}"
                        )
                ACC = acc_t[(i, qi)]
                p2f = prod2.rearrange("p a b -> p (a b)")
                for c0, c1 in _bank_chunks(0, QTR):
                    nc.tensor.matmul(
                        ACC[:, c0:c1],
                        sel_sb["ident"][:],
                        p2f[:, c0:c1],
                        start=(j == 0),
                        stop=(j == KK - 1),
                    )
                if DO_D:
                    for c0, c1 in _bank_chunks(0, QTR):
                        nc.tensor.matmul(
                            d_t[(i, qi)][0:4, c0:c1],
                            sel_sb[s_selD(i)][:],
                            E_u[:, c0:c1],
                            start=(j == 0),
                            stop=(j == KK - 1),
                        )
                if j == KK - 1:
                    emit_norm_out(i, qi)

            def s_selD(i):
                return slices[i]["selD"]

            def emit_norm_out(i, qi):
                s = slices[i]
                ACC = acc_t[(i, qi)]
                out_un = work_pool.tile(
                    [128, QTR], bf, tag="oun", bufs=2, name=f"oun{i}{qi}"
                )
                nc.scalar.copy(out_un[:], ACC[:])
                if DO_D:
                    R_f = small_pool.tile(
                        [4, QTR], fp32, tag="rf", bufs=2, name=f"rf{i}{qi}"
                    )
                    nc.vector.reciprocal_approx_fast(R_f[:], d_t[(i, qi)][0:4, :])
                    R_b = small_pool.tile(
                        [4, QTR], bf, tag="rb", bufs=2, name=f"rb{i}{qi}"
                    )
                    nc.vector.tensor_copy(R_b[:], R_f[:])
                    R_bc = psD.tile([128, QTR], fp32, tag="D", name=f"Rbc{i}{qi}")
                    for c0, c1 in _bank_chunks(0, QTR):
                        nc.tensor.matmul(
                            R_bc[:, c0:c1],
                            sel_sb[s["selNB"]][:],
                            R_b[:, c0:c1],
                            start=True,
                            stop=True,
                        )
                    out_sb = work_pool.tile(
                        [128, QTR], bf, tag="osb", bufs=2, name=f"osb{i}{qi}"
                    )
                    nc.vector.tensor_tensor(out_sb[:], out_un[:], R_bc[:], AL.mult)
                else:
                    out_sb = out_un
                col = s["ocol"] + qi * QTR
                if s["ocn"] == 128:
                    nc.sync.dma_start(o_d[0:128, col : col + QTR], out_sb[:])
                else:
                    nc.sync.dma_start(o_d[128:192, col : col + QTR], out_sb[0:64, :])
                    nc.sync.dma_start(
                        o_d[128:192, HALF + col : HALF + col + QTR],
                        out_sb[64:128, :],
                    )

            for u in units:
                E_u, prod2 = emit_head(*u)
                pending.append((*u, E_u, prod2))
                if len(pending) > TAIL_LAG:
                    emit_tail_ops(*pending.pop(0))
            while pending:
                emit_tail_ops(*pending.pop(0))

    nc.compile()
    return nc, consts


_CACHE = {}


def _get_module():
    if "nc" not in _CACHE:
        _CACHE["nc"], _CACHE["consts"] = build_module()
    return _CACHE["nc"], _CACHE["consts"]


def make_in_maps(q, k, v, consts):
    q = np.asarray(q, np.float32)
    k = np.asarray(k, np.float32)
    v = np.asarray(v, np.float32)
    in_maps = []
    for b in range(B):
        qb = q[b].reshape(C, H, W)
        kb = np.pad(k[b].reshape(C, H, W), ((0, 0), (2, 2), (2, 2)))
        vb = np.pad(v[b].reshape(C, H, W), ((0, 0), (2, 2), (2, 2)))
        m = {
            "q0": np.ascontiguousarray(qb[:128]).astype(BF),
            "q1": np.ascontiguousarray(
                np.concatenate([qb[128:, 0:HROWS], qb[128:, HROWS:H]], axis=0)
            ).astype(BF),
            "k0": np.ascontiguousarray(kb[:128]).astype(BF),
            "v0": np.ascontiguousarray(vb[:128]).astype(BF),
            "k1": np.ascontiguousarray(
                np.concatenate([kb[128:, 0:PADH1], kb[128:, PADH - PADH1 : PADH]], axis=0)
            ).astype(BF),
            "v1": np.ascontiguousarray(
                np.concatenate([vb[128:, 0:PADH1], vb[128:, PADH - PADH1 : PADH]], axis=0)
            ).astype(BF),
        }
        for name, arr in consts.items():
            m[name] = arr.astype(BF)
        in_maps.append(m)
    return in_maps


def kernel(q: np.ndarray, k: np.ndarray, v: np.ndarray) -> np.ndarray:
    from concourse import bass_utils

    nc, consts = _get_module()
    in_maps = make_in_maps(q, k, v, consts)
    res = bass_utils.run_bass_kernel_spmd(nc, in_maps, core_ids=list(range(B)))
    out = np.empty((B, H, W, C), np.float32)
    for b in range(B):
        ob = np.asarray(res.results[b]["o"], dtype=np.float32)  # [C, HWPIX]
        out[b] = ob.reshape(C, H, W).transpose(1, 2, 0)
    return out
